# revision 1
# baseline (speedup 1.0000x reference)
"""Trainium2 Bass kernel for nn_BAR_86045374808446 (sparse_attention).

Math: for each head h (one per NeuronCore, 8 cores):
  s[i,j,d] = ahat_i[d] + bhat_j[d]         (ahat/bhat are d-mean-centered)
  var[i,j] = va[i] + vb[j] + (2/D)<ahat_i, bhat_j>      (matmul!)
  r[i,j]   = 1/sqrt(var + eps)
  out[i,d] = sum_{j<=i} exp(s[i,j,d] * r[i,j])

Factorization (exact to fp32, Taylor order K):
  exp(s*r) = exp(ahat*rbar) * exp(bhat*rbar) * exp(s*w),  w = r - rbar
  exp(s*w) = sum_k (s*w)^k / k! = sum_{p+e=k} w^k * (ahat^p/p!) * (bhat^e/e!)
  => out = sum_p A_p  (*)  sum_e (mask*w^(p+e))^T @ B_e
  with A_p = ahat^p/p! * exp(ahat*rbar)  [i,d],
       B_e = bhat^e/e! * exp(bhat*rbar)  [j,d],
  so the whole T^2*D work is PSUM-accumulated matmuls on the TensorEngine.
"""

import sys

import numpy as np

for _p in ("/opt/trn_rl_repo", "/root/.axon_site/_ro/trn_rl_repo"):
    if _p not in sys.path:
        sys.path.insert(0, _p)

T, D, H, P, NB = 512, 64, 8, 128, 4
K = 11               # taylor order (12 terms)
EPS = 1e-5
CHUNK = (K + 1) * D  # 832 psum cols per i-block

_cached = {}


def _build_nc(use_f32r=True, dump=None):
    import concourse.bass as bass
    import concourse.mybir as mybir
    from concourse import bass_isa
    from concourse.tile import TileContext
    from concourse.masks import make_identity

    f32 = mybir.dt.float32
    f32r = mybir.dt.float32r
    Alu = mybir.AluOpType
    Act = mybir.ActivationFunctionType

    nc = bass.Bass()
    ah_d = nc.declare_dram_parameter("ah", [T, D], f32, isOutput=False)
    bh_d = nc.declare_dram_parameter("bh", [T, D], f32, isOutput=False)
    out_d = nc.declare_dram_parameter("out", [T, D], f32, isOutput=True)
    dbg_d = (nc.declare_dram_parameter("dbg", [P, 4 * T], f32, isOutput=True)
             if dump else None)

    mmdt = f32r if use_f32r else f32

    with TileContext(nc) as tc:
        with (
            tc.tile_pool(name="const", bufs=1) as constp,
            tc.tile_pool(name="work", bufs=1) as work,
            tc.tile_pool(name="wpool", bufs=8) as wpool,
            tc.tile_pool(name="fin", bufs=4) as fin,
            tc.tile_pool(name="psum", bufs=1, space="PSUM") as psum,
        ):
            # ---------------- load ----------------
            Asb = work.tile([P, NB, D], f32, tag="Asb")
            Bsb = work.tile([P, NB, D], f32, tag="Bsb")
            nc.sync.dma_start(out=Asb, in_=ah_d[:].rearrange("(nb p) d -> p nb d", p=P))
            nc.sync.dma_start(out=Bsb, in_=bh_d[:].rearrange("(nb p) d -> p nb d", p=P))

            identity = constp.tile([P, P], f32, tag="ident")
            make_identity(nc, identity)
            eps_col = constp.tile([P, 1], f32, tag="eps")
            nc.vector.memset(eps_col, EPS)
            onesT = constp.tile([P, T], f32, tag="ones")
            nc.gpsimd.memset(onesT, 1.0)
            # warm the ACT Sqrt/Exp tables off the critical path
            warm = constp.tile([P, 1], f32, tag="warm")
            nc.scalar.activation(out=warm, in_=eps_col, func=Act.Sqrt)
            nc.scalar.activation(out=warm, in_=eps_col, func=Act.Exp)

            # ---------------- stats: mean/var per row, center ----------------
            mva = work.tile([P, NB, 2], f32, tag="mva")
            mvb = work.tile([P, NB, 2], f32, tag="mvb")
            A2 = work.tile([P, NB, D], f32, tag="A2")
            Dt = [psum.tile([P, 1024], f32, tag=f"D{ib}", name=f"D{ib}")
                  for ib in range(NB)]
            aT = work.tile([66, NB, P], f32, tag="aT")
            bT = work.tile([66, NB, P], f32, tag="bT")
            for blk in range(NB):
                sa = work.tile([P, 6], f32, tag="bnsA")
                nc.vector.bn_stats(out=sa, in_=Asb[:, blk, :])
                nc.vector.bn_aggr(out=mva[:, blk, :], in_=sa)
                sb = work.tile([P, 6], f32, tag="bnsB")
                nc.vector.bn_stats(out=sb, in_=Bsb[:, blk, :])
                nc.vector.bn_aggr(out=mvb[:, blk, :], in_=sb)
                nc.vector.tensor_scalar(
                    out=Asb[:, blk, :], in0=Asb[:, blk, :],
                    scalar1=mva[:, blk, 0:1], scalar2=None, op0=Alu.subtract)
                nc.vector.tensor_scalar(
                    out=Bsb[:, blk, :], in0=Bsb[:, blk, :],
                    scalar1=mvb[:, blk, 0:1], scalar2=None, op0=Alu.subtract)
                nc.gpsimd.tensor_scalar(out=A2[:, blk, :], in0=Asb[:, blk, :],
                                        scalar1=2.0 / D, scalar2=None,
                                        op0=Alu.mult)
                Ta = work.tile([P, 66], f32, tag="Ta")
                nc.scalar.copy(out=Ta[:, 0:D], in_=A2[:, blk, :])
                nc.gpsimd.memset(Ta[:, D:D + 1], 1.0)
                nc.gpsimd.tensor_copy(out=Ta[:, D + 1:D + 2], in_=mva[:, blk, 1:2])
                Tb = work.tile([P, 66], f32, tag="Tb")
                nc.scalar.copy(out=Tb[:, 0:D], in_=Bsb[:, blk, :])
                nc.gpsimd.tensor_copy(out=Tb[:, D:D + 1], in_=mvb[:, blk, 1:2])
                nc.gpsimd.memset(Tb[:, D + 1:D + 2], 1.0)
                tp = Dt[blk][0:66, 512:512 + P]
                nc.tensor.transpose(tp, Ta, identity)
                nc.vector.tensor_copy(out=aT[:, blk, :], in_=tp)
                tp2 = Dt[blk][0:66, 512 + P:512 + 2 * P]
                nc.tensor.transpose(tp2, Tb, identity)
                nc.vector.tensor_copy(out=bT[:, blk, :], in_=tp2)

            # ---------------- var matmuls -> rT = 1/sqrt(var+eps) -------------
            # varT[j, i] = vb[j] + va[i] + (2/D) sum_d bhatT[d,j] ahatT[d,i]
            rT = work.tile([P, NB, T], f32, tag="rT")
            aT_flat = aT.rearrange("k nb p -> k (nb p)")
            zmx = work.tile([P, NB], f32, tag="zmx")
            zmn = work.tile([P, NB], f32, tag="zmn")
            # m=3 right after m=0 so the global var min/max (-> rbar) is
            # complete two matmuls early and its chain hides under m=1/m=2.
            for m in (0, 3, 1, 2):
                vp = Dt[m][:, 0:T]
                nc.tensor.matmul(vp, bT[:, m, :], aT_flat, start=True, stop=True,
                                 skip_group_check=True)
                nc.vector.tensor_reduce(out=zmx[:, m:m + 1], in_=vp,
                                        axis=mybir.AxisListType.X, op=Alu.max)
                nc.vector.tensor_reduce(out=zmn[:, m:m + 1], in_=vp,
                                        axis=mybir.AxisListType.X, op=Alu.min)
                nc.scalar.activation(out=rT[:, m, :], in_=vp, func=Act.Sqrt,
                                     bias=eps_col, scale=1.0)
                nc.vector.reciprocal(out=rT[:, m, :], in_=rT[:, m, :])
            rT_flat = rT.rearrange("p nb t -> p (nb t)")
            if dump == "r":
                nc.sync.dma_start(out=dbg_d[:], in_=rT_flat)

            # ---------------- rbar, w = r - rbar ------------------------------
            z2 = work.tile([P, 2], f32, tag="z2")
            nc.vector.tensor_reduce(out=z2[:, 0:1], in_=zmx,
                                    axis=mybir.AxisListType.X, op=Alu.max)
            nc.vector.tensor_reduce(out=z2[:, 1:2], in_=zmn,
                                    axis=mybir.AxisListType.X, op=Alu.min)
            nc.vector.tensor_scalar(out=z2[:, 1:2], in0=z2[:, 1:2], scalar1=-1.0,
                                    scalar2=None, op0=Alu.mult)
            # cross-partition: transpose [P,2]->[2,P], reduce free -> [2,1],
            # then rbar = 0.5*max(r) - 0.5*max(-r) broadcast to all partitions
            # via a [2,P] constant matmul (walrus rejects partition_all_reduce).
            ztp = Dt[0][0:2, 768:768 + P]
            nc.tensor.transpose(ztp, z2, identity)
            zrow = work.tile([2, P], f32, tag="zrow")
            nc.vector.tensor_copy(out=zrow, in_=ztp)
            zm = work.tile([2, 1], f32, tag="zm")
            nc.vector.tensor_reduce(out=zm, in_=zrow, axis=mybir.AxisListType.X,
                                    op=Alu.max)
            # zm = [max var, -min var]; restore sign, r = 1/sqrt(v + eps)
            sgn2 = constp.tile([2, 1], f32, tag="sgn2")
            nc.vector.memset(sgn2, 1.0)
            nc.gpsimd.affine_select(out=sgn2, in_=sgn2, compare_op=Alu.is_ge,
                                    fill=-1.0, base=0, channel_multiplier=-1,
                                    pattern=[[0, 1]])
            nc.vector.tensor_scalar(out=zm, in0=zm, scalar1=sgn2, scalar2=None,
                                    op0=Alu.mult)
            nc.scalar.activation(out=zm, in_=zm, func=Act.Sqrt,
                                 bias=eps_col[0:2, :], scale=1.0)
            nc.vector.reciprocal(out=zm, in_=zm)
            half = constp.tile([2, P], f32, tag="half")
            nc.vector.memset(half, 0.5)
            rbp = Dt[1][:, 768:769]
            nc.tensor.matmul(rbp, half, zm, start=True, stop=True,
                             skip_group_check=True)
            rbar = work.tile([P, 1], f32, tag="rbar")
            nc.vector.tensor_copy(out=rbar, in_=rbp)
            # w_m = r_m - rbar (per block, pipelined) and w2_m = w_m^2
            w2 = work.tile([P, NB, T], f32, tag="w2")
            for m in range(NB):
                nc.vector.tensor_scalar(out=rT[:, m, :], in0=rT[:, m, :],
                                        scalar1=rbar, scalar2=None,
                                        op0=Alu.subtract)
                nc.gpsimd.tensor_tensor(out=w2[:, m, P * m:T],
                                        in0=rT[:, m, P * m:T],
                                        in1=rT[:, m, P * m:T], op=Alu.mult)
            if dump == "w":
                nc.sync.dma_start(out=dbg_d[:], in_=rT_flat)
            if dump == "rbar":
                nc.sync.dma_start(out=dbg_d[:, 0:1], in_=rbar)

            # ---------------- A_p, B_e tensors --------------------------------
            # A_all[:, ib, p, :] = ahat^p/p! * exp(ahat*rbar)
            # B_all[:, jb, K-e, :] = bhat^e/e! * exp(bhat*rbar)   (reversed slots)
            A_all = work.tile([P, NB, K + 1, D], f32, tag="A_all")
            B_all = work.tile([P, NB, K + 8, D], mmdt, tag="B_all")
            for nb in range(NB):
                nc.gpsimd.memset(B_all[:, nb, K + 1:K + 8, :].bitcast(f32), 0.0)
            nc.scalar.activation(out=B_all[:, :, K, :], in_=Bsb, func=Act.Exp,
                                 scale=rbar)
            for p_ in range(1, K + 1):
                nc.vector.scalar_tensor_tensor(
                    out=B_all[:, :, K - p_, :], in0=Bsb, scalar=1.0 / p_,
                    in1=B_all[:, :, K - p_ + 1, :], op0=Alu.mult, op1=Alu.mult)
            if dump == "A":
                nc.sync.dma_start(out=dbg_d[:], in_=A_all.rearrange(
                    "p nb k d -> p (nb k d)")[:, 0:4 * T])
            if dump == "B":
                nc.sync.dma_start(out=dbg_d[:], in_=B_all.rearrange(
                    "p nb k d -> p (nb k d)")[:, 0:4 * T].bitcast(f32))

            # ---------------- main loop ----------------------------------------
            def emit_mm(ib, m, k, Wt, last):
                """matmuls for (jblk m, iblock ib, taylor step k).

                start=True resets the whole psum bank, so each bank-region
                gets one full-width start (zero B-slots pad chunks c>k) and
                width-clipped accumulates after that."""
                lhsT = Wt[:, (ib - m) * P:(ib - m) * P + P]
                base = K - k  # slot of chunk c=0
                nseg = (k + 1) * D
                # region A: cols [0, 512) = chunks 0..7
                if m == 0 and k == 0:
                    nc.tensor.matmul(Dt[ib][:, 0:512], lhsT,
                                     B_all[:, m, K:K + 8, :],
                                     start=True, stop=False,
                                     skip_group_check=True)
                else:
                    cA = min(max(nseg, 256), 512)
                    nc.tensor.matmul(Dt[ib][:, 0:cA], lhsT,
                                     B_all[:, m, base:base + cA // D, :],
                                     start=False, stop=last,
                                     skip_group_check=True)
                # region B: cols [512, CHUNK) = chunks 8..12
                if k >= 8:
                    if m == 0 and k == 8:
                        nc.tensor.matmul(Dt[ib][:, 512:CHUNK], lhsT,
                                         B_all[:, m, K:K + (CHUNK - 512) // D, :],
                                         start=True, stop=False,
                                         skip_group_check=True)
                    else:
                        cB = min(max(nseg - 512, 256), CHUNK - 512)
                        nc.tensor.matmul(Dt[ib][:, 512:512 + cB], lhsT,
                                         B_all[:, m, base + 8:base + 8 + cB // D, :],
                                         start=False, stop=last,
                                         skip_group_check=True)

            Wsm = [[None] * (K + 1) for _ in range(NB)]

            def build_w(m, k):
                wm = T - P * m
                Wn = wpool.tile([P, T], mmdt, tag="W", name=f"W{k}_{m}")
                if k == 0:
                    nc.gpsimd.affine_select(
                        out=Wn[:, 0:wm], in_=onesT[:, 0:wm],
                        compare_op=Alu.is_ge, fill=0.0, base=0,
                        channel_multiplier=-1, pattern=[[1, wm]])
                elif k == 1:
                    nc.vector.tensor_tensor(out=Wn[:, 0:wm],
                                            in0=Wsm[m][0][:, 0:wm],
                                            in1=rT[:, m, P * m:T], op=Alu.mult)
                else:
                    # W_k = W_{k-2} * w^2: two chains, split across engines
                    eng = nc.vector if ((k + m) % 2 == 1) else nc.gpsimd
                    eng.tensor_tensor(out=Wn[:, 0:wm],
                                      in0=Wsm[m][k - 2][:, 0:wm],
                                      in1=w2[:, m, P * m:T], op=Alu.mult)
                Wsm[m][k] = Wn

            def emit_final(ib):
                tmp = fin.tile([P, CHUNK], f32, tag="tmp", name=f"tmp{ib}")
                nc.vector.tensor_tensor(out=tmp, in0=A_all[:, ib, :, :],
                                        in1=Dt[ib][:, 0:CHUNK], op=Alu.mult)
                osb = fin.tile([P, D], f32, tag="osb", name=f"osb{ib}")
                nc.vector.tensor_reduce(
                    out=osb, in_=tmp.rearrange("p (s d) -> p d s", s=K + 1),
                    axis=mybir.AxisListType.X, op=Alu.add)
                nc.sync.dma_start(out=out_d[ib * P:(ib + 1) * P, :], in_=osb)

            # m-major: per-jblk k chains; Dt[m] completes at the end of
            # iteration m, so its final is emitted (and runs) right away.
            for m in range(NB):
                for k in range(K + 1):
                    build_w(m, k)
                    for ib in range(m, NB):
                        emit_mm(ib, m, k, Wsm[m][k], last=(m == ib and k == K))
                if m == 0:
                    # A_p tensors (needed only by the finals)
                    nc.scalar.activation(out=A_all[:, :, 0, :], in_=Asb,
                                         func=Act.Exp, scale=rbar)
                    for p_ in range(1, K + 1):
                        nc.vector.scalar_tensor_tensor(
                            out=A_all[:, :, p_, :], in0=Asb, scalar=1.0 / p_,
                            in1=A_all[:, :, p_ - 1, :], op0=Alu.mult,
                            op1=Alu.mult)
                emit_final(m)

            if dump == "D":
                for ib in range(2):
                    dcp = fin.tile([P, CHUNK], f32, tag="dcp", name=f"dcp{ib}")
                    nc.vector.tensor_copy(out=dcp, in_=Dt[ib][:, 0:CHUNK])
                    nc.sync.dma_start(out=dbg_d[:, ib * CHUNK:(ib + 1) * CHUNK],
                                      in_=dcp)


    _split_multi_waits(nc, mybir)
    return nc


def _split_multi_waits(nc, mybir):
    """TRN2 TPB instructions have a single sync-wait slot; walrus cannot
    split >1 wait for several structs. Use the bacc rust pass to split
    them into EventSemaphore instructions."""
    import bass_rust as _bass_rust
    _bass_rust.generate_event_semaphores(nc)
    # walrus rejects wait-only EventSemaphore encodings ("ISA wrong length")
    # and requires update_value == 1. Give each wait-carrier a +1 update of a
    # scratch semaphore nothing ever waits on.
    used = set()
    for f in nc.m.functions:
        for blk in f.blocks:
            for inst in blk.instructions:
                si = getattr(inst, "sync_info", None)
                if si is not None:
                    for w in (si.on_wait or []):
                        used.add(w.id)
                    for u in (si.on_update or []):
                        used.add(u.id)
    scratch = next(s for s in nc._kernel_sem_range if s not in used)
    for f in nc.m.functions:
        for blk in f.blocks:
            for inst in blk.instructions:
                if isinstance(inst, mybir.InstEventSemaphore):
                    si = inst.sync_info
                    if si is not None and si.on_wait and not si.on_update:
                        si.on_update = [_bass_rust.SyncUpdate(
                            sync_type='semaphore', id=scratch,
                            ant_name='wsplit_scratch',
                            update_mode='sem-inc', update_value=1,
                            update_reg=None)]
    # Drop end-of-kernel EVENT_SEMAPHORE_RANGE_CLEAR (opcode 0xb0): this
    # walrus build rejects its encoding ("ISA wrong length"), and the kernel
    # preamble re-clears all kernel semaphores on every run anyway.
    for f in nc.m.functions:
        for blk in f.blocks:
            blk.instructions[:] = [
                inst for inst in blk.instructions
                if not (isinstance(inst, mybir.InstISA)
                        and getattr(inst, "isa_opcode", None) == 0xb0
                        and not (inst.sync_info and
                                 (inst.sync_info.on_wait or
                                  inst.sync_info.on_update)))
            ]


def _get_nc(use_f32r=True, dump=None):
    key = ("nc", use_f32r, dump)
    if key not in _cached:
        _cached[key] = _build_nc(use_f32r, dump)
    return _cached[key]


def kernel(a, b, num_head=8, head_size=64, **kwargs):
    from concourse.bass_utils import run_bass_kernel_spmd

    a = np.asarray(a)
    b = np.asarray(b)
    nc = _get_nc()
    in_maps = []
    for h in range(H):
        in_maps.append({
            "ah": np.ascontiguousarray(a[0, :, h * D:(h + 1) * D], dtype=np.float32),
            "bh": np.ascontiguousarray(b[0, :, h * D:(h + 1) * D], dtype=np.float32),
        })
    res = run_bass_kernel_spmd(nc, in_maps, list(range(H)))
    full = np.concatenate([res.results[h]["out"] for h in range(H)], axis=-1)
    return full[None].astype(np.float32)


if __name__ == "__main__":
    import sys
    sys.path.insert(0, "/opt/trn_rl_repo")
    _build_nc()
    print("build OK")



# revision 17
# speedup vs baseline: 1.7868x; 1.7868x over previous
"""Trainium2 Bass kernel for nn_BAR_86045374808446 (sparse_attention).

Math: for each head h (one per NeuronCore, 8 cores):
  s[i,j,d] = ahat_i[d] + bhat_j[d]         (ahat/bhat are d-mean-centered)
  var[i,j] = va[i] + vb[j] + (2/D)<ahat_i, bhat_j>      (matmul!)
  r[i,j]   = 1/sqrt(var + eps)
  out[i,d] = sum_{j<=i} exp(s[i,j,d] * r[i,j])

Factorization (Taylor around rbar, r = rbar + w):
  exp(s*r) = exp(ahat*rbar) * exp(bhat*rbar) * exp(s*w)
  exp(s*w) = sum_k (s*w)^k / k! = sum_{c+e=k} w^k * (ahat^c/c!) * (bhat^e/e!)
  => out = sum_c A_c (*) sum_e (mask*w^(c+e))^T @ B_e      [c<=CM, e<=EM, c+e<=K]
  with A_c = ahat^c/c! * exp(ahat*rbar)  [i,d],
       B_e = bhat^e/e! * exp(bhat*rbar)  [j,d],
so the T^2*D work is PSUM-accumulated bf16 matmuls on the TensorEngine.
Tolerance is 2e-2; numerics sims put this config at ~6e-4.

rbar is centered from row stats only: r in [rsqrt(max va + max vb + eps),
rsqrt(min va + min vb + eps)], midpoint. Cheap (no T^2 reductions) and the
Taylor order has ample margin for the slightly off-center choice.
"""

import sys

import numpy as np

for _p in ("/opt/trn_rl_repo", "/root/.axon_site/_ro/trn_rl_repo"):
    if _p not in sys.path:
        sys.path.insert(0, _p)

T, D, H, P, NB = 512, 64, 8, 128, 4
K = 6                # total Taylor order (c + e <= K)
CM = 5               # max A-side power (psum chunks 0..CM)
EM = 5               # max B-side power
NSLOT = EM + 1 + CM  # B slots: s in [0,EM] holds B_{EM-s}; s>EM are zeros
FCH = (CM + 1) * D   # final/psum width per i-block
EPS = 1e-5

_cached = {}


def _build_nc(dump=None):
    import concourse.bass as bass
    import concourse.mybir as mybir
    from concourse.tile import TileContext
    from concourse.masks import make_identity

    f32 = mybir.dt.float32
    f32r = mybir.dt.float32r
    bf16 = mybir.dt.bfloat16
    Alu = mybir.AluOpType
    Act = mybir.ActivationFunctionType

    nc = bass.Bass()
    ah_d = nc.declare_dram_parameter("ah", [T, D], f32, isOutput=False)
    bh_d = nc.declare_dram_parameter("bh", [T, D], f32, isOutput=False)
    out_d = nc.declare_dram_parameter("out", [T, D], f32, isOutput=True)
    dbg_d = (nc.declare_dram_parameter("dbg", [P, 4 * T], f32, isOutput=True)
             if dump else None)

    with TileContext(nc) as tc:
        with (
            tc.tile_pool(name="const", bufs=1) as constp,
            tc.tile_pool(name="work", bufs=1) as work,
            tc.tile_pool(name="wpool", bufs=4) as wpool,
            tc.tile_pool(name="mpool", bufs=2) as mpool,
            tc.tile_pool(name="fin", bufs=4) as fin,
            tc.tile_pool(name="psA", bufs=1, space="PSUM") as psA,
            tc.tile_pool(name="psV", bufs=2, space="PSUM") as psV,
            tc.tile_pool(name="psT", bufs=2, space="PSUM") as psT,
        ):
            # ---------------- loads + constants ----------------
            Asb = work.tile([P, NB, D], f32, tag="Asb")
            Bsb = work.tile([P, NB, D], f32, tag="Bsb")
            nc.sync.dma_start(out=Asb, in_=ah_d[:].rearrange("(nb p) d -> p nb d", p=P))
            nc.sync.dma_start(out=Bsb, in_=bh_d[:].rearrange("(nb p) d -> p nb d", p=P))

            identity = constp.tile([P, P], f32, tag="ident")
            make_identity(nc, identity)
            eps_col = constp.tile([P, 1], f32, tag="eps")
            nc.vector.memset(eps_col, EPS)
            onesT = constp.tile([P, T], bf16, tag="ones")
            nc.gpsimd.memset(onesT, 1.0)
            # causal mask W0[j, c] = (c >= j); same [P, wm] prefix for every
            # j-block (c is the i-offset within the block's column window)
            W0 = constp.tile([P, T], bf16, tag="W0")
            nc.gpsimd.affine_select(
                out=W0, in_=onesT, compare_op=Alu.is_ge, fill=0.0,
                base=0, channel_multiplier=-1, pattern=[[1, T]])
            # rbar combine consts (full-tile memsets only; walrus rejects
            # offset memsets)
            halfP = constp.tile([1, P], f32, tag="halfP")
            nc.vector.memset(halfP, 0.5)
            # warm the ACT Sqrt/Exp tables off the critical path
            warm = constp.tile([P, 1], f32, tag="warm")
            nc.scalar.activation(out=warm, in_=eps_col, func=Act.Sqrt)
            nc.scalar.activation(out=warm, in_=eps_col, func=Act.Exp)

            # B slots EM+1.. are zero pads for the one full-width psum start
            B_all = work.tile([P, NB, NSLOT, D], bf16, tag="B_all")
            nc.gpsimd.memset(B_all[:, :, EM + 1:NSLOT, :], 0.0)

            # ---------------- stats + augmented transposes ----------------
            # Raw-vector gram trick (no WAR hazards, no offset memsets):
            #   Ta = [2/D*a_raw | 1 | mua | va],  Tb = [b_raw | vb | -2*mub | 1]
            #   dot over 67 rows = (2/D)<a,b> + vb - 2*mua*mub + va
            #                    = va + vb + (2/D)<ahat,bhat> = var
            # The trailing 1s come from full-tile memsets before the fills.
            Ta = work.tile([P, NB, 67], f32, tag="Ta")
            Tb = work.tile([P, NB, 67], f32, tag="Tb")
            mvb = work.tile([P, NB, 2], f32, tag="mvb")
            nc.gpsimd.memset(Ta, 1.0)
            nc.gpsimd.memset(Tb, 1.0)
            aT = work.tile([67, NB, P], f32r, tag="aT")
            bT = work.tile([67, NB, P], f32r, tag="bT")
            def _copy_v(out, in_):
                nc.vector.tensor_copy(out=out, in_=in_)

            def _copy_g(out, in_):
                nc.gpsimd.tensor_copy(out=out, in_=in_)

            def _copy_s(out, in_):
                nc.scalar.copy(out=out, in_=in_)

            # gpsimd cannot read PSUM; rotate psum->sbuf copies DVE/Act
            cp_eng = [_copy_v, _copy_s, _copy_v, _copy_s,
                      _copy_v, _copy_s, _copy_v, _copy_s]
            for blk in range(NB):
                sa = mpool.tile([P, 6], f32, tag="bnsA", name=f"bnsA{blk}")
                nc.vector.bn_stats(out=sa, in_=Asb[:, blk, :])
                nc.vector.bn_aggr(out=Ta[:, blk, 65:67], in_=sa)
                sb = mpool.tile([P, 6], f32, tag="bnsB", name=f"bnsB{blk}")
                nc.vector.bn_stats(out=sb, in_=Bsb[:, blk, :])
                nc.vector.bn_aggr(out=mvb[:, blk, :], in_=sb)
                # raw-value rows for the gram matmul (read before centering)
                nc.scalar.mul(Ta[:, blk, 0:64], Asb[:, blk, :], 2.0 / D)
                nc.scalar.copy(out=Tb[:, blk, 0:64], in_=Bsb[:, blk, :])
                nc.gpsimd.tensor_copy(out=Tb[:, blk, 64:65],
                                      in_=mvb[:, blk, 1:2])
                nc.vector.tensor_scalar(
                    out=Tb[:, blk, 65:66], in0=mvb[:, blk, 0:1],
                    scalar1=-2.0, scalar2=None, op0=Alu.mult)
                # center in place
                nc.vector.tensor_scalar(
                    out=Asb[:, blk, :], in0=Asb[:, blk, :],
                    scalar1=Ta[:, blk, 65:66], scalar2=None, op0=Alu.subtract)
                nc.vector.tensor_scalar(
                    out=Bsb[:, blk, :], in0=Bsb[:, blk, :],
                    scalar1=mvb[:, blk, 0:1], scalar2=None, op0=Alu.subtract)
                tpa = psT.tile([P, P], f32, tag="tp", name=f"tpa{blk}")
                nc.tensor.transpose(tpa[0:67, :], Ta[:, blk, :], identity)
                cp_eng[2 * blk](aT[:, blk, :], tpa[0:67, :])
                tpb = psT.tile([P, P], f32, tag="tp", name=f"tpb{blk}")
                nc.tensor.transpose(tpb[0:67, :], Tb[:, blk, :], identity)
                cp_eng[2 * blk + 1](bT[:, blk, :], tpb[0:67, :])

            # ---------------- rbar from row stats ----------------
            # z4 = [max va | max vb | min va | min vb] per partition
            z4 = work.tile([P, 4], f32, tag="z4")
            nc.vector.tensor_reduce(out=z4[:, 0:1], in_=Ta[:, :, 66],
                                    axis=mybir.AxisListType.X, op=Alu.max)
            nc.vector.tensor_reduce(out=z4[:, 1:2], in_=mvb[:, :, 1],
                                    axis=mybir.AxisListType.X, op=Alu.max)
            nc.vector.tensor_reduce(out=z4[:, 2:3], in_=Ta[:, :, 66],
                                    axis=mybir.AxisListType.X, op=Alu.min)
            nc.vector.tensor_reduce(out=z4[:, 3:4], in_=mvb[:, :, 1],
                                    axis=mybir.AxisListType.X, op=Alu.min)
            nc.vector.tensor_scalar(out=z4[:, 2:4], in0=z4[:, 2:4],
                                    scalar1=-1.0, scalar2=None, op0=Alu.mult)
            ztp = psT.tile([P, P], f32, tag="tp", name="ztp")
            nc.tensor.transpose(ztp[0:4, :], z4, identity)
            zrow = work.tile([4, P], f32, tag="zrow")
            nc.vector.tensor_copy(out=zrow, in_=ztp[0:4, :])
            g4 = work.tile([4, 1], f32, tag="g4")
            nc.vector.tensor_reduce(out=g4, in_=zrow,
                                    axis=mybir.AxisListType.X, op=Alu.max)
            # bring all four onto partition 0, pair-sum to [vmax, -vmin]
            gtp = psT.tile([P, P], f32, tag="tp", name="gtp")
            nc.tensor.transpose(gtp[0:1, 0:4], g4, identity[0:4, 0:4])
            grow = work.tile([1, 4], f32, tag="grow")
            nc.vector.tensor_copy(out=grow, in_=gtp[0:1, 0:4])
            vhl = work.tile([1, 2], f32, tag="vhl")
            nc.vector.tensor_reduce(
                out=vhl, in_=grow.rearrange("p (g t) -> p g t", g=2),
                axis=mybir.AxisListType.X, op=Alu.add)
            nc.vector.tensor_scalar(out=vhl[0:1, 1:2], in0=vhl[0:1, 1:2],
                                    scalar1=-1.0, scalar2=None, op0=Alu.mult)
            rhl = work.tile([1, 2], f32, tag="rhl")
            nc.scalar.activation(out=rhl, in_=vhl, func=Act.Sqrt,
                                 bias=eps_col[0:1, :], scale=1.0)
            nc.vector.reciprocal(out=rhl, in_=rhl)
            rsum = work.tile([1, 1], f32, tag="rsum")
            nc.vector.tensor_reduce(out=rsum, in_=rhl,
                                    axis=mybir.AxisListType.X, op=Alu.add)
            rb_p = psT.tile([P, P], f32, tag="tp", name="rbp")
            nc.tensor.matmul(rb_p[:, 0:1], halfP, rsum, start=True, stop=True,
                             skip_group_check=True)
            rbar = work.tile([P, 1], f32, tag="rbar")
            nc.vector.tensor_copy(out=rbar, in_=rb_p[:, 0:1])
            if dump == "rbar":
                nc.sync.dma_start(out=dbg_d[:, 0:1], in_=rbar)

            # ---------------- A_c, B_e tensors ----------------
            nc.scalar.activation(out=B_all[:, :, EM, :], in_=Bsb, func=Act.Exp,
                                 scale=rbar)
            for e in range(1, EM + 1):
                nc.vector.scalar_tensor_tensor(
                    out=B_all[:, :, EM - e, :], in0=Bsb, scalar=1.0 / e,
                    in1=B_all[:, :, EM - e + 1, :], op0=Alu.mult, op1=Alu.mult)
            A_all = work.tile([P, NB, CM + 1, D], f32, tag="A_all")
            nc.scalar.activation(out=A_all[:, :, 0, :], in_=Asb, func=Act.Exp,
                                 scale=rbar)
            for c in range(1, CM + 1):
                nc.vector.scalar_tensor_tensor(
                    out=A_all[:, :, c, :], in0=Asb, scalar=1.0 / c,
                    in1=A_all[:, :, c - 1, :], op0=Alu.mult, op1=Alu.mult)

            # ---------------- per-block r, w, w^2 ----------------
            aT_flat = aT.rearrange("k nb p -> k (nb p)")
            Dt = [psA.tile([P, FCH], f32, tag=f"D{ib}", name=f"D{ib}")
                  for ib in range(NB)]
            wv = [None] * NB
            w2v = [None] * NB
            for m in range(NB):
                wm = T - P * m
                vp = psV.tile([P, 512], f32, tag="vp", name=f"vp{m}")
                nc.tensor.matmul(vp[:, 0:wm], bT[:, m, :], aT_flat[:, P * m:T],
                                 start=True, stop=True, skip_group_check=True)
                sq = mpool.tile([P, T], f32, tag="sq", name=f"sq{m}")
                nc.scalar.activation(out=sq[:, 0:wm], in_=vp[:, 0:wm],
                                     func=Act.Sqrt, bias=eps_col, scale=1.0)
                rw = mpool.tile([P, T], bf16, tag="rw", name=f"rw{m}")
                with nc.allow_low_precision(reason="bf16 r is within tolerance"):
                    nc.vector.reciprocal(out=rw[:, 0:wm], in_=sq[:, 0:wm])
                wt = mpool.tile([P, T], bf16, tag="wv", name=f"wv{m}")
                nc.vector.tensor_scalar(out=wt[:, 0:wm], in0=rw[:, 0:wm],
                                        scalar1=rbar, scalar2=None,
                                        op0=Alu.subtract)
                w2t = mpool.tile([P, T], bf16, tag="w2", name=f"w2{m}")
                nc.vector.tensor_tensor(out=w2t[:, 0:wm], in0=wt[:, 0:wm],
                                        in1=wt[:, 0:wm], op=Alu.mult)
                wv[m] = wt
                w2v[m] = w2t
            if dump == "r":
                rdbg = work.tile([P, NB, T], f32, tag="rdbg")
                nc.gpsimd.memset(rdbg, 0.0)
                for m in range(NB):
                    nc.vector.tensor_copy(out=rdbg[:, m, 0:T - P * m],
                                          in_=wv[m][:, 0:T - P * m])
                nc.sync.dma_start(
                    out=dbg_d[:], in_=rdbg.rearrange("p nb t -> p (nb t)"))

            # ---------------- main loop ----------------
            def emit_mm(ib, m, k, Wt, last):
                c_lo = max(0, k - EM)
                c_hi = min(k, CM)
                lhsT = Wt[:, (ib - m) * P:(ib - m) * P + P]
                if m == 0 and k == 0:
                    # one full-width start zeroes the whole bank region;
                    # pad slots EM+1.. are zeros
                    nc.tensor.matmul(Dt[ib][:, 0:FCH], lhsT,
                                     B_all[:, m, EM:EM + CM + 1, :],
                                     start=True, stop=False,
                                     skip_group_check=True)
                else:
                    s_lo = EM - k + c_lo
                    nchunk = c_hi - c_lo + 1
                    nc.tensor.matmul(Dt[ib][:, c_lo * D:(c_hi + 1) * D], lhsT,
                                     B_all[:, m, s_lo:s_lo + nchunk, :],
                                     start=False, stop=last,
                                     skip_group_check=True)

            def emit_final(ib):
                tmp = fin.tile([P, FCH], f32, tag="tmp", name=f"tmp{ib}")
                nc.vector.tensor_tensor(out=tmp, in0=A_all[:, ib, :, :],
                                        in1=Dt[ib][:, 0:FCH], op=Alu.mult)
                osb = fin.tile([P, D], f32, tag="osb", name=f"osb{ib}")
                nc.vector.tensor_reduce(
                    out=osb, in_=tmp.rearrange("p (s d) -> p d s", s=CM + 1),
                    axis=mybir.AxisListType.X, op=Alu.add)
                nc.sync.dma_start(out=out_d[ib * P:(ib + 1) * P, :], in_=osb)

            Wsm = {}
            for m in range(NB):
                wm = T - P * m
                for k in range(K + 1):
                    if k == 0:
                        Wt = W0
                    else:
                        Wt = wpool.tile([P, T], bf16, tag="W", name=f"W{k}_{m}")
                        if k == 1:
                            nc.vector.tensor_tensor(
                                out=Wt[:, 0:wm], in0=W0[:, 0:wm],
                                in1=wv[m][:, 0:wm], op=Alu.mult)
                        else:
                            prev = Wsm[(m, k - 2)]
                            nc.vector.tensor_tensor(
                                out=Wt[:, 0:wm], in0=prev[:, 0:wm],
                                in1=w2v[m][:, 0:wm], op=Alu.mult)
                    Wsm[(m, k)] = Wt
                    for ib in range(m, NB):
                        emit_mm(ib, m, k, Wt, last=(m == ib and k == K))
                emit_final(m)

            if dump == "D":
                for ib in range(2):
                    dcp = fin.tile([P, FCH], f32, tag="dcp", name=f"dcp{ib}")
                    nc.vector.tensor_copy(out=dcp, in_=Dt[ib][:, 0:FCH])
                    nc.sync.dma_start(out=dbg_d[:, ib * FCH:(ib + 1) * FCH],
                                      in_=dcp)

    _split_multi_waits(nc, mybir)
    return nc


def _split_multi_waits(nc, mybir):
    """TRN2 TPB instructions have a single sync-wait slot; walrus cannot
    split >1 wait for several structs. Use the bacc rust pass to split
    them into EventSemaphore instructions."""
    import bass_rust as _bass_rust
    _bass_rust.generate_event_semaphores(nc)
    # walrus rejects wait-only EventSemaphore encodings ("ISA wrong length")
    # and requires update_value == 1. Give each wait-carrier a +1 update of a
    # scratch semaphore nothing ever waits on.
    used = set()
    for f in nc.m.functions:
        for blk in f.blocks:
            for inst in blk.instructions:
                si = getattr(inst, "sync_info", None)
                if si is not None:
                    for w in (si.on_wait or []):
                        used.add(w.id)
                    for u in (si.on_update or []):
                        used.add(u.id)
    scratch = next(s for s in nc._kernel_sem_range if s not in used)
    for f in nc.m.functions:
        for blk in f.blocks:
            for inst in blk.instructions:
                if isinstance(inst, mybir.InstEventSemaphore):
                    si = inst.sync_info
                    if si is not None and si.on_wait and not si.on_update:
                        si.on_update = [_bass_rust.SyncUpdate(
                            sync_type='semaphore', id=scratch,
                            ant_name='wsplit_scratch',
                            update_mode='sem-inc', update_value=1,
                            update_reg=None)]
    # Drop end-of-kernel EVENT_SEMAPHORE_RANGE_CLEAR (opcode 0xb0): this
    # walrus build rejects its encoding ("ISA wrong length"), and the kernel
    # preamble re-clears all kernel semaphores on every run anyway.
    for f in nc.m.functions:
        for blk in f.blocks:
            blk.instructions[:] = [
                inst for inst in blk.instructions
                if not (isinstance(inst, mybir.InstISA)
                        and getattr(inst, "isa_opcode", None) == 0xb0
                        and not (inst.sync_info and
                                 (inst.sync_info.on_wait or
                                  inst.sync_info.on_update)))
            ]


def _get_nc(dump=None):
    key = ("nc", dump)
    if key not in _cached:
        _cached[key] = _build_nc(dump)
    return _cached[key]


def kernel(a, b, num_head=8, head_size=64, **kwargs):
    from concourse.bass_utils import run_bass_kernel_spmd

    a = np.asarray(a)
    b = np.asarray(b)
    nc = _get_nc()
    in_maps = []
    for h in range(H):
        in_maps.append({
            "ah": np.ascontiguousarray(a[0, :, h * D:(h + 1) * D], dtype=np.float32),
            "bh": np.ascontiguousarray(b[0, :, h * D:(h + 1) * D], dtype=np.float32),
        })
    res = run_bass_kernel_spmd(nc, in_maps, list(range(H)))
    full = np.concatenate([res.results[h]["out"] for h in range(H)], axis=-1)
    return full[None].astype(np.float32)


if __name__ == "__main__":
    import sys
    sys.path.insert(0, "/opt/trn_rl_repo")
    _build_nc()
    print("build OK")


# revision 27
# speedup vs baseline: 2.1139x; 1.1831x over previous
"""Trainium2 Bass kernel for nn_BAR_86045374808446 (sparse_attention).

Math: for each head h (one per NeuronCore, 8 cores):
  s[i,j,d] = ahat_i[d] + bhat_j[d]         (ahat/bhat are d-mean-centered)
  var[i,j] = va[i] + vb[j] + (2/D)<ahat_i, bhat_j>      (matmul!)
  r[i,j]   = 1/sqrt(var + eps)
  out[i,d] = sum_{j<=i} exp(s[i,j,d] * r[i,j])

Factorization (Taylor around rbar, r = rbar + w):
  exp(s*r) = exp(ahat*rbar) * exp(bhat*rbar) * exp(s*w)
  exp(s*w) = sum_k (s*w)^k / k! = sum_{c+e=k} w^k * (ahat^c/c!) * (bhat^e/e!)
  => out = sum_c A_c (*) sum_e (mask*w^(c+e))^T @ B_e      [c<=CM, e<=EM, c+e<=K]
  with A_c = ahat^c/c! * exp(ahat*rbar)  [i,d],
       B_e = bhat^e/e! * exp(bhat*rbar)  [j,d],
so the T^2*D work is PSUM-accumulated bf16 matmuls on the TensorEngine.
Tolerance is 2e-2; numerics sims put this config at ~6e-4.

rbar is centered from row stats only: r in [rsqrt(max va + max vb + eps),
rsqrt(min va + min vb + eps)], midpoint. Cheap (no T^2 reductions) and the
Taylor order has ample margin for the slightly off-center choice.
"""

import sys

import numpy as np

for _p in ("/opt/trn_rl_repo", "/root/.axon_site/_ro/trn_rl_repo"):
    if _p not in sys.path:
        sys.path.insert(0, _p)

T, D, H, P, NB = 512, 64, 8, 128, 4
K = 5                # total Taylor order (c + e <= K)
CM = 4               # max A-side power (psum chunks 0..CM)
EM = 4               # max B-side power
NSLOT = EM + 1 + CM  # B slots: s in [0,EM] holds B_{EM-s}; s>EM are zeros
FCH = (CM + 1) * D   # final/psum width per i-block
EPS = 1e-5

_cached = {}


def _build_nc(dump=None):
    import concourse.bass as bass
    import concourse.mybir as mybir
    from concourse.tile import TileContext
    from concourse.masks import make_identity

    f32 = mybir.dt.float32
    f32r = mybir.dt.float32r
    bf16 = mybir.dt.bfloat16
    Alu = mybir.AluOpType
    Act = mybir.ActivationFunctionType

    nc = bass.Bass()
    ah_d = nc.declare_dram_parameter("ah", [T, D], f32, isOutput=False)
    bh_d = nc.declare_dram_parameter("bh", [T, D], f32, isOutput=False)
    out_d = nc.declare_dram_parameter("out", [T, D], f32, isOutput=True)
    dbg_d = (nc.declare_dram_parameter("dbg", [P, 4 * T], f32, isOutput=True)
             if dump else None)

    with TileContext(nc) as tc:
        with (
            tc.tile_pool(name="const", bufs=1) as constp,
            tc.tile_pool(name="work", bufs=1) as work,
            tc.tile_pool(name="wpool", bufs=4) as wpool,
            tc.tile_pool(name="mpool", bufs=2) as mpool,
            tc.tile_pool(name="fin", bufs=4) as fin,
            tc.tile_pool(name="psA", bufs=1, space="PSUM") as psA,
            tc.tile_pool(name="psV", bufs=2, space="PSUM") as psV,
            tc.tile_pool(name="psT", bufs=2, space="PSUM") as psT,
        ):
            # ---------------- loads + constants ----------------
            Asb = work.tile([P, NB, D], f32, tag="Asb")
            Bsb = work.tile([P, NB, D], f32, tag="Bsb")
            nc.sync.dma_start(out=Asb, in_=ah_d[:].rearrange("(nb p) d -> p nb d", p=P))
            nc.sync.dma_start(out=Bsb, in_=bh_d[:].rearrange("(nb p) d -> p nb d", p=P))

            identity = constp.tile([P, P], f32, tag="ident")
            make_identity(nc, identity)
            eps_col = constp.tile([P, 1], f32, tag="eps")
            nc.vector.memset(eps_col, EPS)
            onesT = constp.tile([P, T], bf16, tag="ones")
            nc.gpsimd.memset(onesT, 1.0)
            # causal mask W0[j, c] = (c >= j); same [P, wm] prefix for every
            # j-block (c is the i-offset within the block's column window)
            W0 = constp.tile([P, T], bf16, tag="W0")
            nc.gpsimd.affine_select(
                out=W0, in_=onesT, compare_op=Alu.is_ge, fill=0.0,
                base=0, channel_multiplier=-1, pattern=[[1, T]])
            # rbar combine consts (full-tile memsets only; walrus rejects
            # offset memsets)
            halfP = constp.tile([1, P], f32, tag="halfP")
            nc.vector.memset(halfP, 0.5)
            # warm the ACT Sqrt/Exp tables off the critical path
            warm = constp.tile([P, 1], f32, tag="warm")
            nc.scalar.activation(out=warm, in_=eps_col, func=Act.Sqrt)
            nc.scalar.activation(out=warm, in_=eps_col, func=Act.Exp)

            # B slots EM+1.. are zero pads for the one full-width psum start
            B_all = work.tile([P, NB, NSLOT, D], bf16, tag="B_all")
            nc.gpsimd.memset(B_all[:, :, EM + 1:NSLOT, :], 0.0)

            # ---------------- stats + augmented transposes ----------------
            # Raw-vector gram trick (no WAR hazards, no offset memsets):
            #   Ta = [2/D*a_raw | 1 | mua | va],  Tb = [b_raw | vb | -2*mub | 1]
            #   dot over 67 rows = (2/D)<a,b> + vb - 2*mua*mub + va
            #                    = va + vb + (2/D)<ahat,bhat> = var
            # The trailing 1s come from full-tile memsets before the fills.
            Ta = work.tile([P, NB, 67], f32, tag="Ta")
            Tb = work.tile([P, NB, 67], f32, tag="Tb")
            mvb = work.tile([P, NB, 2], f32, tag="mvb")
            nc.gpsimd.memset(Ta, 1.0)
            nc.gpsimd.memset(Tb, 1.0)
            aT = work.tile([67, NB, P], f32r, tag="aT")
            bT = work.tile([67, NB, P], f32r, tag="bT")
            def _copy_v(out, in_):
                nc.vector.tensor_copy(out=out, in_=in_)

            def _copy_g(out, in_):
                nc.gpsimd.tensor_copy(out=out, in_=in_)

            def _copy_s(out, in_):
                nc.scalar.copy(out=out, in_=in_)

            # gpsimd cannot read PSUM; rotate psum->sbuf copies DVE/Act
            cp_eng = [_copy_s, _copy_v, _copy_s, _copy_s,
                      _copy_v, _copy_s, _copy_s, _copy_v]
            for blk in range(NB):
                sa = mpool.tile([P, 6], f32, tag="bnsA", name=f"bnsA{blk}")
                nc.vector.bn_stats(out=sa, in_=Asb[:, blk, :])
                nc.vector.bn_aggr(out=Ta[:, blk, 65:67], in_=sa)
                sb = mpool.tile([P, 6], f32, tag="bnsB", name=f"bnsB{blk}")
                nc.vector.bn_stats(out=sb, in_=Bsb[:, blk, :])
                nc.vector.bn_aggr(out=mvb[:, blk, :], in_=sb)
                # raw-value rows for the gram matmul (read before centering)
                nc.scalar.mul(Ta[:, blk, 0:64], Asb[:, blk, :], 2.0 / D)
                nc.scalar.copy(out=Tb[:, blk, 0:64], in_=Bsb[:, blk, :])
                nc.gpsimd.tensor_copy(out=Tb[:, blk, 64:65],
                                      in_=mvb[:, blk, 1:2])
                nc.vector.tensor_scalar(
                    out=Tb[:, blk, 65:66], in0=mvb[:, blk, 0:1],
                    scalar1=-2.0, scalar2=None, op0=Alu.mult)
                # center in place
                nc.vector.tensor_scalar(
                    out=Asb[:, blk, :], in0=Asb[:, blk, :],
                    scalar1=Ta[:, blk, 65:66], scalar2=None, op0=Alu.subtract)
                nc.vector.tensor_scalar(
                    out=Bsb[:, blk, :], in0=Bsb[:, blk, :],
                    scalar1=mvb[:, blk, 0:1], scalar2=None, op0=Alu.subtract)
                tpa = psT.tile([P, P], f32, tag="tp", name=f"tpa{blk}")
                nc.tensor.transpose(tpa[0:67, :], Ta[:, blk, :], identity)
                cp_eng[2 * blk](aT[:, blk, :], tpa[0:67, :])
                tpb = psT.tile([P, P], f32, tag="tp", name=f"tpb{blk}")
                nc.tensor.transpose(tpb[0:67, :], Tb[:, blk, :], identity)
                cp_eng[2 * blk + 1](bT[:, blk, :], tpb[0:67, :])

            # ---------------- rbar from row stats ----------------
            # z4 = [max va | max vb | min va | min vb] per partition
            z4 = work.tile([P, 4], f32, tag="z4")
            nc.vector.tensor_reduce(out=z4[:, 0:1], in_=Ta[:, :, 66],
                                    axis=mybir.AxisListType.X, op=Alu.max)
            nc.vector.tensor_reduce(out=z4[:, 1:2], in_=mvb[:, :, 1],
                                    axis=mybir.AxisListType.X, op=Alu.max)
            nc.vector.tensor_reduce(out=z4[:, 2:3], in_=Ta[:, :, 66],
                                    axis=mybir.AxisListType.X, op=Alu.min)
            nc.vector.tensor_reduce(out=z4[:, 3:4], in_=mvb[:, :, 1],
                                    axis=mybir.AxisListType.X, op=Alu.min)
            nc.vector.tensor_scalar(out=z4[:, 2:4], in0=z4[:, 2:4],
                                    scalar1=-1.0, scalar2=None, op0=Alu.mult)
            ztp = psT.tile([P, P], f32, tag="tp", name="ztp")
            nc.tensor.transpose(ztp[0:4, :], z4, identity)
            zrow = work.tile([4, P], f32, tag="zrow")
            nc.vector.tensor_copy(out=zrow, in_=ztp[0:4, :])
            g4 = work.tile([4, 1], f32, tag="g4")
            nc.vector.tensor_reduce(out=g4, in_=zrow,
                                    axis=mybir.AxisListType.X, op=Alu.max)
            # bring all four onto partition 0, pair-sum to [vmax, -vmin]
            gtp = psT.tile([P, P], f32, tag="tp", name="gtp")
            nc.tensor.transpose(gtp[0:1, 0:4], g4, identity[0:4, 0:4])
            grow = work.tile([1, 4], f32, tag="grow")
            nc.vector.tensor_copy(out=grow, in_=gtp[0:1, 0:4])
            vhl = work.tile([1, 2], f32, tag="vhl")
            nc.vector.tensor_reduce(
                out=vhl, in_=grow.rearrange("p (g t) -> p g t", g=2),
                axis=mybir.AxisListType.X, op=Alu.add)
            nc.vector.tensor_scalar(out=vhl[0:1, 1:2], in0=vhl[0:1, 1:2],
                                    scalar1=-1.0, scalar2=None, op0=Alu.mult)
            rhl = work.tile([1, 2], f32, tag="rhl")
            nc.scalar.activation(out=rhl, in_=vhl, func=Act.Sqrt,
                                 bias=eps_col[0:1, :], scale=1.0)
            nc.vector.reciprocal(out=rhl, in_=rhl)
            rsum = work.tile([1, 1], f32, tag="rsum")
            nc.vector.tensor_reduce(out=rsum, in_=rhl,
                                    axis=mybir.AxisListType.X, op=Alu.add)
            rb_p = psT.tile([P, P], f32, tag="tp", name="rbp")
            nc.tensor.matmul(rb_p[:, 0:1], halfP, rsum, start=True, stop=True,
                             skip_group_check=True)
            rbar = work.tile([P, 1], f32, tag="rbar")
            nc.vector.tensor_copy(out=rbar, in_=rb_p[:, 0:1])
            if dump == "rbar":
                nc.sync.dma_start(out=dbg_d[:, 0:1], in_=rbar)

            # ---------------- A_c, B_e tensors ----------------
            nc.scalar.activation(out=B_all[:, :, EM, :], in_=Bsb, func=Act.Exp,
                                 scale=rbar)
            for e in range(1, EM + 1):
                nc.vector.scalar_tensor_tensor(
                    out=B_all[:, :, EM - e, :], in0=Bsb, scalar=1.0 / e,
                    in1=B_all[:, :, EM - e + 1, :], op0=Alu.mult, op1=Alu.mult)
            # A chain off DVE: Act makes the step tensors (ahat/c), Pool
            # multiplies the chain (gpsimd TT is SBUF-only but that's fine)
            A_all = work.tile([P, NB, CM + 1, D], f32, tag="A_all")
            stepA = work.tile([P, CM, NB, D], f32, tag="stepA")
            for c in range(1, CM + 1):
                nc.scalar.mul(stepA[:, c - 1, :, :], Asb, 1.0 / c)
            nc.scalar.activation(out=A_all[:, :, 0, :], in_=Asb, func=Act.Exp,
                                 scale=rbar)
            for c in range(1, CM + 1):
                nc.gpsimd.tensor_tensor(
                    out=A_all[:, :, c, :], in0=A_all[:, :, c - 1, :],
                    in1=stepA[:, c - 1, :, :], op=Alu.mult)

            # ---------------- per-block r, w, w^2 ----------------
            aT_flat = aT.rearrange("k nb p -> k (nb p)")
            Dt = [psA.tile([P, FCH], f32, tag=f"D{ib}", name=f"D{ib}")
                  for ib in range(NB)]
            wv = [None] * NB
            w2v = [None] * NB
            for m in range(NB):
                wm = T - P * m
                vp = psV.tile([P, 512], f32, tag="vp", name=f"vp{m}")
                nc.tensor.matmul(vp[:, 0:wm], bT[:, m, :], aT_flat[:, P * m:T],
                                 start=True, stop=True, skip_group_check=True)
                # r = 1/sqrt(v+eps) = exp(-0.5*ln(v+eps)); both funcs live in
                # one act table set, and this keeps the rsqrt off the DVE
                lnv = mpool.tile([P, T], f32, tag="lnv", name=f"lnv{m}")
                nc.scalar.activation(out=lnv[:, 0:wm], in_=vp[:, 0:wm],
                                     func=Act.Ln, bias=eps_col, scale=1.0)
                rw = mpool.tile([P, T], bf16, tag="rw", name=f"rw{m}")
                nc.scalar.activation(out=rw[:, 0:wm], in_=lnv[:, 0:wm],
                                     func=Act.Exp, scale=-0.5)
                wt = mpool.tile([P, T], bf16, tag="wv", name=f"wv{m}")
                nc.vector.tensor_scalar(out=wt[:, 0:wm], in0=rw[:, 0:wm],
                                        scalar1=rbar, scalar2=None,
                                        op0=Alu.subtract)
                w2t = mpool.tile([P, T], bf16, tag="w2", name=f"w2{m}")
                nc.vector.tensor_tensor(out=w2t[:, 0:wm], in0=wt[:, 0:wm],
                                        in1=wt[:, 0:wm], op=Alu.mult)
                wv[m] = wt
                w2v[m] = w2t
            if dump == "r":
                rdbg = work.tile([P, NB, T], f32, tag="rdbg")
                nc.gpsimd.memset(rdbg, 0.0)
                for m in range(NB):
                    nc.vector.tensor_copy(out=rdbg[:, m, 0:T - P * m],
                                          in_=wv[m][:, 0:T - P * m])
                nc.sync.dma_start(
                    out=dbg_d[:], in_=rdbg.rearrange("p nb t -> p (nb t)"))

            # ---------------- main loop ----------------
            def emit_mm(ib, m, k, Wt, last):
                c_lo = max(0, k - EM)
                c_hi = min(k, CM)
                lhsT = Wt[:, (ib - m) * P:(ib - m) * P + P]
                if m == 0 and k == 0:
                    # one full-width start zeroes the whole bank region;
                    # pad slots EM+1.. are zeros
                    nc.tensor.matmul(Dt[ib][:, 0:FCH], lhsT,
                                     B_all[:, m, EM:EM + CM + 1, :],
                                     start=True, stop=False,
                                     skip_group_check=True)
                else:
                    s_lo = EM - k + c_lo
                    nchunk = c_hi - c_lo + 1
                    nc.tensor.matmul(Dt[ib][:, c_lo * D:(c_hi + 1) * D], lhsT,
                                     B_all[:, m, s_lo:s_lo + nchunk, :],
                                     start=False, stop=last,
                                     skip_group_check=True)

            def emit_final(ib):
                tmp = fin.tile([P, FCH], f32, tag="tmp", name=f"tmp{ib}")
                nc.vector.tensor_tensor(out=tmp, in0=A_all[:, ib, :, :],
                                        in1=Dt[ib][:, 0:FCH], op=Alu.mult)
                osb = fin.tile([P, D], f32, tag="osb", name=f"osb{ib}")
                nc.vector.tensor_reduce(
                    out=osb, in_=tmp.rearrange("p (s d) -> p d s", s=CM + 1),
                    axis=mybir.AxisListType.X, op=Alu.add)
                nc.sync.dma_start(out=out_d[ib * P:(ib + 1) * P, :], in_=osb)

            Wsm = {}
            for m in range(NB):
                wm = T - P * m
                for k in range(K + 1):
                    if k == 0:
                        Wt = W0
                    else:
                        Wt = wpool.tile([P, T], bf16, tag="W", name=f"W{k}_{m}")
                        weng = nc.gpsimd if m == 3 else nc.vector
                        if k == 1:
                            weng.tensor_tensor(
                                out=Wt[:, 0:wm], in0=W0[:, 0:wm],
                                in1=wv[m][:, 0:wm], op=Alu.mult)
                        else:
                            prev = Wsm[(m, k - 2)]
                            weng.tensor_tensor(
                                out=Wt[:, 0:wm], in0=prev[:, 0:wm],
                                in1=w2v[m][:, 0:wm], op=Alu.mult)
                    Wsm[(m, k)] = Wt
                    for ib in range(m, NB):
                        emit_mm(ib, m, k, Wt, last=(m == ib and k == K))
                emit_final(m)

            if dump == "D":
                for ib in range(2):
                    dcp = fin.tile([P, FCH], f32, tag="dcp", name=f"dcp{ib}")
                    nc.vector.tensor_copy(out=dcp, in_=Dt[ib][:, 0:FCH])
                    nc.sync.dma_start(out=dbg_d[:, ib * FCH:(ib + 1) * FCH],
                                      in_=dcp)

    _split_multi_waits(nc, mybir)
    return nc


def _split_multi_waits(nc, mybir):
    """TRN2 TPB instructions have a single sync-wait slot; walrus cannot
    split >1 wait for several structs. Use the bacc rust pass to split
    them into EventSemaphore instructions."""
    import bass_rust as _bass_rust
    _bass_rust.generate_event_semaphores(nc)
    # walrus rejects wait-only EventSemaphore encodings ("ISA wrong length")
    # and requires update_value == 1. Give each wait-carrier a +1 update of a
    # scratch semaphore nothing ever waits on.
    used = set()
    for f in nc.m.functions:
        for blk in f.blocks:
            for inst in blk.instructions:
                si = getattr(inst, "sync_info", None)
                if si is not None:
                    for w in (si.on_wait or []):
                        used.add(w.id)
                    for u in (si.on_update or []):
                        used.add(u.id)
    scratch = next(s for s in nc._kernel_sem_range if s not in used)
    for f in nc.m.functions:
        for blk in f.blocks:
            for inst in blk.instructions:
                if isinstance(inst, mybir.InstEventSemaphore):
                    si = inst.sync_info
                    if si is not None and si.on_wait and not si.on_update:
                        si.on_update = [_bass_rust.SyncUpdate(
                            sync_type='semaphore', id=scratch,
                            ant_name='wsplit_scratch',
                            update_mode='sem-inc', update_value=1,
                            update_reg=None)]
    # Drop end-of-kernel EVENT_SEMAPHORE_RANGE_CLEAR (opcode 0xb0): this
    # walrus build rejects its encoding ("ISA wrong length"), and the kernel
    # preamble re-clears all kernel semaphores on every run anyway.
    for f in nc.m.functions:
        for blk in f.blocks:
            blk.instructions[:] = [
                inst for inst in blk.instructions
                if not (isinstance(inst, mybir.InstISA)
                        and getattr(inst, "isa_opcode", None) == 0xb0
                        and not (inst.sync_info and
                                 (inst.sync_info.on_wait or
                                  inst.sync_info.on_update)))
            ]


def _get_nc(dump=None):
    key = ("nc", dump)
    if key not in _cached:
        _cached[key] = _build_nc(dump)
    return _cached[key]


def kernel(a, b, num_head=8, head_size=64, **kwargs):
    from concourse.bass_utils import run_bass_kernel_spmd

    a = np.asarray(a)
    b = np.asarray(b)
    nc = _get_nc()
    in_maps = []
    for h in range(H):
        in_maps.append({
            "ah": np.ascontiguousarray(a[0, :, h * D:(h + 1) * D], dtype=np.float32),
            "bh": np.ascontiguousarray(b[0, :, h * D:(h + 1) * D], dtype=np.float32),
        })
    res = run_bass_kernel_spmd(nc, in_maps, list(range(H)))
    full = np.concatenate([res.results[h]["out"] for h in range(H)], axis=-1)
    return full[None].astype(np.float32)


if __name__ == "__main__":
    import sys
    sys.path.insert(0, "/opt/trn_rl_repo")
    _build_nc()
    print("build OK")
